# revision 9
# baseline (speedup 1.0000x reference)
"""DICNN-CRF forward kernel for 8 Trainium2 NeuronCores.

Data-parallel over the batch: 64 examples -> 8 per core.  Per core the whole
activation set lives in SBUF in feature-major layout [128 features, 8192
tokens] (fp16).  Convs/linears are TensorE matmuls over 512-token chunks with
PSUM fp32 accumulation; channel-LayerNorm stats are computed with selector
matmuls that pack per-chunk token statistics into a [16, 512] PSUM tile,
rstd = exp(-0.5*ln(var+eps)) on ScalarE, and the per-token scale/shift are
broadcast back across the 128 feature partitions with K=16 selector matmuls.
The embedding gather is an indirect DMA (token-major) followed by xbar DMA
transposes into feature-major.
"""
import os
import sys

if "/opt/trn_rl_repo" not in sys.path:
    sys.path.insert(0, "/opt/trn_rl_repo")

import numpy as np

N_CORES = 8
B, L, V, E, F, H, T = 64, 1024, 32000, 128, 128, 256, 20
BPC = B // N_CORES         # examples per core
NT = BPC * L               # tokens per core (8192)
CH = 512                   # chunk (tokens) per matmul
NCH = NT // CH             # 16 chunks
PAD = 2                    # conv halo (max dilation)
LW = L + 2 * PAD
NTILE = NT // 128          # 64 token-tiles (gather/output)
EPS = 1e-5

_CACHE: dict = {}


def _build_program():
    import concourse.bass as bass
    import concourse.tile as tile
    from concourse import bacc, mybir
    from concourse import library_config

    dt = mybir.dt
    Alu = mybir.AluOpType
    Act = mybir.ActivationFunctionType

    TRUNC = int(os.environ.get("KTRUNC", "999"))
    _step = [0]

    def gate():
        _step[0] += 1
        return _step[0] <= TRUNC

    nc = bacc.Bacc("TRN2", target_bir_lowering=False, debug=False,
                   enable_asserts=False, num_devices=N_CORES)

    # ---- DRAM I/O ----
    d_idx = nc.dram_tensor("idx", [128, NTILE], dt.int32, kind="ExternalInput").ap()
    d_emb = nc.dram_tensor("emb", [V, F], dt.float16, kind="ExternalInput").ap()
    d_lw = [nc.dram_tensor(f"lw{i}", [F, F], dt.float16, kind="ExternalInput").ap()
            for i in range(4)]
    d_lb = [nc.dram_tensor(f"lb{i}", [F, 1], dt.float32, kind="ExternalInput").ap()
            for i in range(4)]
    d_cw = {}
    d_cb = {}
    for i in range(4):
        for cn in "abd":
            d_cw[(i, cn)] = nc.dram_tensor(f"cw{i}{cn}", [F, 3, F], dt.float16,
                                           kind="ExternalInput").ap()
            d_cb[(i, cn)] = nc.dram_tensor(f"cb{i}{cn}", [F, 1], dt.float32,
                                           kind="ExternalInput").ap()
    d_h1w = nc.dram_tensor("h1w", [F, 2, F], dt.float16, kind="ExternalInput").ap()
    d_h1b = nc.dram_tensor("h1b", [F, 2], dt.float32, kind="ExternalInput").ap()
    d_h2w = nc.dram_tensor("h2w", [F, 2, T], dt.float16, kind="ExternalInput").ap()
    d_h2b = nc.dram_tensor("h2b", [T, 1], dt.float32, kind="ExternalInput").ap()
    d_selm = nc.dram_tensor("selm", [F, NCH, NCH], dt.float16, kind="ExternalInput").ap()
    d_selb = nc.dram_tensor("selb", [NCH, NCH, F], dt.float16, kind="ExternalInput").ap()
    d_eye = nc.dram_tensor("eye20", [T, T], dt.float32, kind="ExternalInput").ap()
    d_out = nc.dram_tensor("out", [NT, T], dt.float32, kind="ExternalOutput").ap()

    with tile.TileContext(nc) as tc:
        with (
            tc.tile_pool(name="consts", bufs=1) as consts,
            tc.tile_pool(name="acts", bufs=1) as acts,
            tc.tile_pool(name="small", bufs=4) as small,
            tc.tile_pool(name="psc", bufs=2, space="PSUM") as psc,
            tc.tile_pool(name="pss", bufs=2, space="PSUM") as pss,
            tc.tile_pool(name="psb", bufs=4, space="PSUM") as psb,
        ):
            nc.gpsimd.load_library(library_config.mlp)

            # ---- load constants into SBUF ----
            def cload(name, dram, shape, dty):
                t = consts.tile(shape, dty, name=name)
                nc.sync.dma_start(t[:], dram)
                return t

            s_lw = [cload(f"s_lw{i}", d_lw[i], [F, F], dt.float16) for i in range(4)]
            s_lb = [cload(f"s_lb{i}", d_lb[i], [F, 1], dt.float32) for i in range(4)]
            s_cw = {k: cload(f"s_cw{k[0]}{k[1]}", v, [F, 3, F], dt.float16)
                    for k, v in d_cw.items()}
            s_cb = {k: cload(f"s_cb{k[0]}{k[1]}", v, [F, 1], dt.float32)
                    for k, v in d_cb.items()}
            s_h1w = cload("s_h1w", d_h1w, [F, 2, F], dt.float16)
            s_h1b = cload("s_h1b", d_h1b, [F, 2], dt.float32)
            s_h2w = cload("s_h2w", d_h2w, [F, 2, T], dt.float16)
            s_h2b = cload("s_h2b", d_h2b, [T, 1], dt.float32)
            s_selm = cload("s_selm", d_selm, [F, NCH, NCH], dt.float16)
            s_selb = cload("s_selb", d_selb, [NCH, NCH, F], dt.float16)
            s_eye = cload("s_eye", d_eye, [T, T], dt.float32)
            s_idx = cload("s_idx", d_idx, [128, NTILE], dt.int32)
            s_eps = consts.tile([NCH, 1], dt.float32, name="s_eps")
            nc.vector.memset(s_eps[:], EPS)

            # ---- embedding gather (token-major) + transpose to feature-major
            gtm = acts.tile([128, NTILE, F], dt.float16, name="gtm", tag="uflat",
                            bufs=2)
            if gate():
                for j in range(NTILE):
                    nc.gpsimd.indirect_dma_start(
                        out=gtm[:, j, :], out_offset=None,
                        in_=d_emb,
                        in_offset=bass.IndirectOffsetOnAxis(ap=s_idx[:, j:j + 1], axis=0),
                    )
            else:
                nc.vector.memset(gtm[:], 0.0)
            h0 = acts.tile([128, NT], dt.float16, name="h0", tag="ynp", bufs=3)
            if gate():
                for j in range(NTILE):
                    nc.sync.dma_start_transpose(h0[:, j * 128:(j + 1) * 128], gtm[:, j, :])
            else:
                nc.vector.memset(h0[:], 0.0)

            # ---- helpers ----
            def new_pad(name):
                t = acts.tile([128, BPC, LW], dt.float16, name=name, tag="xpad",
                              bufs=3)
                nc.vector.memset(t[:, :, 0:PAD], 0.0)
                nc.vector.memset(t[:, :, PAD + L:LW], 0.0)
                return t

            def pad_chunk(t, c):
                e, hh = divmod(c, 2)
                return t[:, e, PAD + hh * CH: PAD + hh * CH + CH]

            def pad_slice(t, c, s):
                e, hh = divmod(c, 2)
                o = PAD + hh * CH + s
                return t[:, e, o:o + CH]

            def flat_chunk(t, c):
                return t[:, c * CH:(c + 1) * CH]

            stage_idx = [0]

            def ln_apply(u, y_dst, y_chunk):
                """LayerNorm over channels of u [128, NT] -> y via y_chunk(y_dst, c)."""
                si = stage_idx[0]
                if not gate():
                    for c in range(NCH):
                        nc.vector.tensor_copy(y_chunk(y_dst, c), flat_chunk(u, c))
                    stage_idx[0] += 1
                    return
                mean_ps = pss.tile([NCH, CH], dt.float32, name=f"mean{si}",
                                   tag="mean", bufs=1)
                msq_ps = pss.tile([NCH, CH], dt.float32, name=f"msq{si}",
                                  tag="msq", bufs=1)
                for c in range(NCH):
                    uc = flat_chunk(u, c)
                    nc.tensor.matmul(mean_ps[:], s_selm[:, c, :], uc,
                                     start=(c == 0), stop=(c == NCH - 1))
                    sqc = small.tile([128, CH], dt.float16, name=f"sq{si}_{c}",
                                     tag="sq")
                    nc.vector.tensor_mul(sqc[:], uc, uc)
                    nc.tensor.matmul(msq_ps[:], s_selm[:, c, :], sqc[:],
                                     start=(c == 0), stop=(c == NCH - 1))
                mean_sb = small.tile([NCH, CH], dt.float32, name=f"mn_{si}", tag="st0")
                nc.vector.tensor_copy(mean_sb[:], mean_ps[:])
                nmean2 = small.tile([NCH, CH], dt.float32, name=f"nm2_{si}", tag="st1")
                nc.vector.scalar_tensor_tensor(
                    out=nmean2[:], in0=mean_sb[:], scalar=-1.0, in1=mean_sb[:],
                    op0=Alu.mult, op1=Alu.mult)
                var = small.tile([NCH, CH], dt.float32, name=f"var_{si}", tag="st2")
                nc.vector.tensor_add(var[:], msq_ps[:], nmean2[:])
                lnv = small.tile([NCH, CH], dt.float32, name=f"lnv_{si}", tag="st3")
                nc.scalar.activation(lnv[:], var[:], Act.Ln, bias=s_eps[:])
                a_sb = small.tile([NCH, CH], dt.float16, name=f"a_{si}", tag="st4")
                nc.scalar.activation(a_sb[:], lnv[:], Act.Exp, scale=-0.5)
                b_sb = small.tile([NCH, CH], dt.float16, name=f"b_{si}", tag="st5")
                nc.vector.scalar_tensor_tensor(
                    out=b_sb[:], in0=mean_sb[:], scalar=-1.0, in1=a_sb[:],
                    op0=Alu.mult, op1=Alu.mult)
                for c in range(NCH):
                    a_ps = psb.tile([128, CH], dt.float32, name=f"aps{si}_{c}",
                                    tag="bc")
                    nc.tensor.matmul(a_ps[:], s_selb[:, c, :], a_sb[:])
                    b_ps = psb.tile([128, CH], dt.float32, name=f"bps{si}_{c}",
                                    tag="bc")
                    nc.tensor.matmul(b_ps[:], s_selb[:, c, :], b_sb[:])
                    t1 = small.tile([128, CH], dt.float16, name=f"t1_{si}_{c}",
                                    tag="ap1")
                    nc.vector.scalar_tensor_tensor(
                        out=t1[:], in0=flat_chunk(u, c), scalar=0.0, in1=a_ps[:],
                        op0=Alu.bypass, op1=Alu.mult)
                    nc.vector.tensor_add(y_chunk(y_dst, c), t1[:], b_ps[:])
                stage_idx[0] += 1

            def mm_stage(src, src_slice, taps, bias, relu, dst, dst_chunk):
                """dst = [relu](sum_k lhsT_k.T @ src_shift_k + bias)."""
                si = stage_idx[0]
                if not gate():
                    for c in range(NCH):
                        nc.vector.tensor_copy(dst_chunk(dst, c), src_slice(src, c, 0))
                    return
                for c in range(NCH):
                    ps = psc.tile([128, CH], dt.float32, name=f"ps{si}_{c}",
                                  tag="mm")
                    for k, (wap, s) in enumerate(taps):
                        nc.tensor.matmul(ps[:], wap, src_slice(src, c, s),
                                         start=(k == 0), stop=(k == len(taps) - 1))
                    if relu:
                        nc.vector.tensor_scalar(
                            out=dst_chunk(dst, c), in0=ps[:], scalar1=bias,
                            scalar2=0.0, op0=Alu.add, op1=Alu.max)
                    else:
                        nc.vector.tensor_scalar(
                            out=dst_chunk(dst, c), in0=ps[:], scalar1=bias,
                            scalar2=None, op0=Alu.add)

            # ---- the 4 layers ----
            h = h0
            h_slice = lambda t, c, s: flat_chunk(t, c)
            for i in range(4):
                xa = new_pad(f"xa{i}")
                mm_stage(h, h_slice, [(s_lw[i][:], 0)], s_lb[i][:], False,
                         xa, pad_chunk)
                ua = acts.tile([128, NT], dt.float16, name=f"ua{i}", tag="uflat",
                               bufs=2)
                mm_stage(xa, pad_slice,
                         [(s_cw[(i, 'a')][:, k, :], k - 1) for k in range(3)],
                         s_cb[(i, 'a')][:], True, ua, flat_chunk)
                y1 = new_pad(f"y1_{i}")
                ln_apply(ua, y1, pad_chunk)

                ub = acts.tile([128, NT], dt.float16, name=f"ub{i}", tag="uflat",
                               bufs=2)
                mm_stage(y1, pad_slice,
                         [(s_cw[(i, 'b')][:, k, :], k - 1) for k in range(3)],
                         s_cb[(i, 'b')][:], True, ub, flat_chunk)
                y2 = new_pad(f"y2_{i}")
                ln_apply(ub, y2, pad_chunk)

                ud = acts.tile([128, NT], dt.float16, name=f"ud{i}", tag="uflat",
                               bufs=2)
                mm_stage(y2, pad_slice,
                         [(s_cw[(i, 'd')][:, k, :], 2 * (k - 1)) for k in range(3)],
                         s_cb[(i, 'd')][:], True, ud, flat_chunk)
                y3 = acts.tile([128, NT], dt.float16, name=f"y3_{i}", tag="ynp",
                               bufs=3)
                ln_apply(ud, y3, flat_chunk)

                u4 = acts.tile([128, NT], dt.float16, name=f"u4_{i}", tag="uflat",
                               bufs=2)
                for c in range(NCH):
                    nc.vector.tensor_scalar(
                        out=flat_chunk(u4, c), in0=flat_chunk(y3, c), scalar1=0.0,
                        scalar2=None, op0=Alu.max)
                y4 = acts.tile([128, NT], dt.float16, name=f"y4_{i}", tag="ynp",
                               bufs=3)
                ln_apply(u4, y4, flat_chunk)
                h = y4

            # ---- heads ----
            h1lo = acts.tile([128, NT], dt.float16, name="h1lo", tag="uflat", bufs=2)
            h1hi = acts.tile([128, NT], dt.float16, name="h1hi", tag="uflat", bufs=2)
            for half, dstt in ((0, h1lo), (1, h1hi)):
                si = stage_idx[0]
                for c in range(NCH):
                    ps = psc.tile([128, CH], dt.float32, name=f"hps{si}_{half}_{c}",
                                  tag="mm")
                    nc.tensor.matmul(ps[:], s_h1w[:, half, :], flat_chunk(h, c))
                    nc.vector.tensor_scalar(
                        out=flat_chunk(dstt, c), in0=ps[:],
                        scalar1=s_h1b[:, half:half + 1], scalar2=None, op0=Alu.add)
            stage_idx[0] += 1

            out_tm = acts.tile([128, NTILE, T], dt.float32, name="out_tm")
            for c in range(NCH):
                ps2 = psc.tile([T, CH], dt.float32, name=f"h2ps_{c}", tag="mm")
                nc.tensor.matmul(ps2[:], s_h2w[:, 0, :], flat_chunk(h1lo, c),
                                 start=True, stop=False)
                nc.tensor.matmul(ps2[:], s_h2w[:, 1, :], flat_chunk(h1hi, c),
                                 start=False, stop=True)
                h2c = small.tile([T, CH], dt.float32, name=f"h2c_{c}", tag="h2c")
                nc.vector.tensor_scalar(
                    out=h2c[:], in0=ps2[:], scalar1=s_h2b[:], scalar2=None,
                    op0=Alu.add)
                for q in range(4):
                    j = c * 4 + q
                    tp = psb.tile([128, T], dt.float32, name=f"tp_{j}", tag="bc")
                    nc.tensor.transpose(tp[:], h2c[:, q * 128:(q + 1) * 128],
                                        s_eye[:])
                    nc.vector.tensor_copy(out_tm[:, j, :], tp[:])

            out_view = d_out.rearrange("(j p) c -> p j c", p=128)
            nc.sync.dma_start(out_view, out_tm[:])

    nc.compile()
    return nc


def _prep_maps(inputs, params):
    idx_all = np.asarray(inputs).astype(np.int32)          # (B, L)
    P = lambda a: np.asarray(a, dtype=np.float32)
    f16 = np.float16

    emb = P(params["emb"]).astype(f16)                     # [V, F]
    base = {"emb": emb}
    for i in range(4):
        base[f"lw{i}"] = P(params["lin"][i]["W"]).astype(f16)
        base[f"lb{i}"] = P(params["lin"][i]["b"]).reshape(F, 1)
        blk = params["blocks"][i]
        for cn in "abd":
            w = P(blk["w" + cn])                           # (O, I, K)
            # lhsT per tap: [I, K, O] -> tensor [F, 3, F]
            base[f"cw{i}{cn}"] = np.ascontiguousarray(
                w.transpose(1, 2, 0)).astype(f16)
            base[f"cb{i}{cn}"] = P(blk["b" + cn]).reshape(F, 1)
    h1w = P(params["head1"]["W"])                          # [F, 2H?] -> [F,256]
    base["h1w"] = np.ascontiguousarray(h1w.reshape(F, 2, F)).astype(f16)
    base["h1b"] = P(params["head1"]["b"]).reshape(2, F).T.copy()   # [F, 2]
    h2w = P(params["head2"]["W"])                          # [256, 20]
    base["h2w"] = np.ascontiguousarray(h2w.reshape(2, F, T).transpose(1, 0, 2)).astype(f16)
    base["h2b"] = P(params["head2"]["b"]).reshape(T, 1)
    selm = np.zeros((F, NCH, NCH), np.float16)
    for c in range(NCH):
        selm[:, c, c] = 1.0 / 128.0
    base["selm"] = selm
    selb = np.zeros((NCH, NCH, F), np.float16)
    for c in range(NCH):
        selb[c, c, :] = 1.0
    base["selb"] = selb
    base["eye20"] = np.eye(T, dtype=np.float32)

    maps = []
    for core in range(N_CORES):
        toks = idx_all[core * BPC:(core + 1) * BPC].reshape(NT)     # g = e*L + l
        idx32 = toks.reshape(NTILE, 128).T.copy()                   # [128, NTILE]
        m = dict(base)
        m["idx"] = idx32
        maps.append(m)
    return maps


def kernel(inputs, params):
    from concourse.bass_utils import run_bass_kernel_spmd

    if "nc" not in _CACHE:
        _CACHE["nc"] = _build_program()
    nc = _CACHE["nc"]
    maps = _prep_maps(inputs, params)
    kw = {}
    td = os.environ.get("KERNEL_TMPDIR")
    if td:
        os.makedirs(td, exist_ok=True)
        kw["tmpdir"] = td
    res = run_bass_kernel_spmd(nc, maps, core_ids=list(range(N_CORES)), **kw)
    _CACHE["last_results"] = res
    out = np.concatenate(
        [res.results[c]["out"].reshape(BPC, L, T) for c in range(N_CORES)], axis=0)
    return out.astype(np.float32)
